# revision 13
# baseline (speedup 1.0000x reference)
"""Multi-head attention (B=4, S=2048, D=1024, H=16, causal) on 8 TRN2 NeuronCores.

Sharding: core = (batch b, head-group g) with 4 batches x 2 groups of 8 heads.
Each core computes Q/K/V projections for its 512 output dims, flash-style
causal attention for its 8 heads, and a partial out-projection over its 512
contraction dims. Host sums the two partials per batch (the row-parallel
all-reduce) and adds the folded biases (Wo @ bv + bo).

All matmuls run in float32r (fp32 storage, FP22 multiply at full PE rate).

Device layouts (T suffix = transposed so the contraction dim sits on SBUF
partitions):
  xqT/xkT/xvT [1024, 2048]   q/k/v[b].T
  wqT/wkT/wvT [1024, 512]    W[g-rows].T (wq pre-scaled by 1/sqrt(64))
  woT         [512, 1024]    Wo[:, g-cols].T
  maskT       [128, 896]     maskT[kk, c] = (c >= kk + 384); sliding windows of
                             this give all 4 diagonal-straddle masks in [k, q]
  bq2/bk2     [128, 4]       per-partition bias per o-block
Output: out [2048, 1024] partial (no bias) in fp32.
"""

import numpy as np

import concourse.bass as bass
import concourse.tile as tile
from concourse import bacc, mybir
from concourse.bass_utils import run_bass_kernel_spmd

F32 = mybir.dt.float32
F32R = mybir.dt.float32r

B, S, D, H = 4, 2048, 1024, 16
DK = 64
O = 512               # per-core qkv output dims (8 heads x 64)
NCORE = 8
NKB = S // 128        # 16 k-blocks
NQU = S // 512        # 4 q-units
NOB = O // 128        # 4 o-blocks
NDB = D // 128        # 8 d-blocks
NCH = S // 256        # 8 t-chunks for projection streaming


def build_mha_program(causal: bool):
    nc = bacc.Bacc("TRN2", target_bir_lowering=False, debug=False)

    xqT = nc.dram_tensor("xqT", [D, S], F32R, kind="ExternalInput")
    xkT = nc.dram_tensor("xkT", [D, S], F32R, kind="ExternalInput")
    xvT = nc.dram_tensor("xvT", [D, S], F32R, kind="ExternalInput")
    wqT = nc.dram_tensor("wqT", [D, O], F32R, kind="ExternalInput")
    wkT = nc.dram_tensor("wkT", [D, O], F32R, kind="ExternalInput")
    wvT = nc.dram_tensor("wvT", [D, O], F32R, kind="ExternalInput")
    woT = nc.dram_tensor("woT", [O, D], F32R, kind="ExternalInput")
    bq2 = nc.dram_tensor("bq2", [128, NOB], F32, kind="ExternalInput")
    bk2 = nc.dram_tensor("bk2", [128, NOB], F32, kind="ExternalInput")
    maskT = nc.dram_tensor("maskT", [128, 896], F32R, kind="ExternalInput")
    onesd = nc.dram_tensor("onesd", [128, 64], F32R, kind="ExternalInput")
    out = nc.dram_tensor("out", [S, D], F32, kind="ExternalOutput")

    with tile.TileContext(nc) as tc:
        with (
            tc.tile_pool(name="wts", bufs=2) as wts,
            tc.tile_pool(name="xin", bufs=2) as xin,
            tc.tile_pool(name="qkv", bufs=1) as qkv,
            tc.tile_pool(name="ctxp", bufs=1) as ctxp,
            tc.tile_pool(name="pt", bufs=3) as ptp,
            tc.tile_pool(name="small", bufs=1) as small,
            tc.tile_pool(name="rbuf", bufs=2) as rbuf,
            tc.tile_pool(name="outst", bufs=2) as outst,
            tc.tile_pool(name="ps_proj", bufs=2, space="PSUM") as ps_proj,
            tc.tile_pool(name="ps_s", bufs=2, space="PSUM") as ps_s,
            tc.tile_pool(name="ps_ctx", bufs=2, space="PSUM") as ps_ctx,
        ):
            # ---- constants ----
            # row 64 is the K=1 stationary operand for the recip broadcast
            ones_sb = small.tile([128, 64], F32R, tag="ones_sb")
            nc.sync.dma_start(out=ones_sb, in_=onesd[:, :])
            bq_sb = small.tile([128, NOB], F32, tag="bq")
            nc.sync.dma_start(out=bq_sb, in_=bq2[:, :])
            bk_sb = small.tile([128, NOB], F32, tag="bk")
            nc.sync.dma_start(out=bk_sb, in_=bk2[:, :])
            if causal:
                mask_sb = small.tile([128, 896], F32R, tag="mask")
                nc.sync.dma_start(out=mask_sb, in_=maskT[:, :])

            # ---- persistent activation tensors ----
            qT_sb = qkv.tile([128, NOB, S], F32R, tag="qT")   # [p, ob, t]
            kT_sb = qkv.tile([128, NOB, S], F32R, tag="kT")   # [p, ob, t]
            # v with a ones column per head: lhsT [V_h | 1] gives ctx rows
            # 0..63 plus the softmax denominator at row 64 in one matmul
            v_sb = qkv.tile([128, NKB, 8, DK + 1], F32R, tag="v")
            ones_col = bass.AP(
                tensor=onesd, offset=0,
                ap=[[64, 128], [0, NKB * 8], [0, 1]],
            )
            nc.sync.dma_start(
                out=v_sb[:, :, :, DK : DK + 1].rearrange("p a b c -> p (a b) c"),
                in_=ones_col,
            )
            ctx_sb = ctxp.tile([128, NOB, S], F32R, tag="ctx")  # [c, cb, q]

            # ================= Phase P: projections =================
            xT_dram = {"q": xqT, "k": xkT, "v": xvT}

            def proj_pass(which):
                w_dram = {"q": wqT, "k": wkT, "v": wvT}[which]
                w_sb = wts.tile([128, NDB, O], F32R, tag="w")
                nc.sync.dma_start(
                    out=w_sb,
                    in_=w_dram[:, :].rearrange("(db p) o -> p db o", p=128),
                )
                x_re = xT_dram[which][:, :].rearrange("(db p) t -> p db t", p=128)
                for ch in range(NCH):
                    tsl = slice(256 * ch, 256 * ch + 256)
                    x_sb = xin.tile([128, NDB, 256], F32R, tag="x")
                    nc.sync.dma_start(out=x_sb, in_=x_re[:, :, tsl])
                    if which in ("q", "k"):
                        dst = qT_sb if which == "q" else kT_sb
                        bias = bq_sb if which == "q" else bk_sb
                        for ob in range(NOB):
                            ps = ps_proj.tile([128, 256], F32, tag="proj")
                            for db in range(NDB):
                                nc.tensor.matmul(
                                    ps,
                                    w_sb[:, db, 128 * ob : 128 * ob + 128],
                                    x_sb[:, db, :],
                                    start=(db == 0),
                                    stop=(db == NDB - 1),
                                )
                            nc.vector.tensor_scalar_add(
                                out=dst[:, ob, tsl],
                                in0=ps,
                                scalar1=bias[:, ob : ob + 1],
                            )
                    else:
                        for tb2 in range(2):
                            tb = 2 * ch + tb2
                            ps = ps_proj.tile([128, O], F32, tag="proj")
                            for db in range(NDB):
                                nc.tensor.matmul(
                                    ps,
                                    x_sb[:, db, 128 * tb2 : 128 * tb2 + 128],
                                    w_sb[:, db, :],
                                    start=(db == 0),
                                    stop=(db == NDB - 1),
                                )
                            nc.vector.tensor_copy(
                                out=v_sb[:, tb, :, 0:DK],
                                in_=ps.rearrange("p (h c) -> p h c", c=DK),
                            )

            proj_pass("q")
            proj_pass("k")
            proj_pass("v")

            # ================= Phase A: attention =================
            for g in range(NQU):
                qsl = slice(512 * g, 512 * g + 512)
                nkb = 4 * g + 4 if causal else NKB
                for hp in range(4):  # head pair: heads 2hp, 2hp+1
                    ctx_ps = [
                        ps_ctx.tile([128, 512], F32, tag="ctx", name=f"ctx_ps{m}")
                        for m in range(2)
                    ]
                    for grp in range(nkb // 2):
                        s_ps = [
                            ps_s.tile([128, 2, 512], F32, tag="s", name=f"s_ps{m}")
                            for m in range(2)
                        ]
                        p_sb = [
                            ptp.tile([128, 2, 512], F32R, tag="p", name=f"p_sb{m}")
                            for m in range(2)
                        ]
                        for j in range(2):
                            kb = 2 * grp + j
                            for m in range(2):  # head in pair (row-packed)
                                psl = slice(64 * m, 64 * m + 64)
                                nc.tensor.matmul(
                                    s_ps[m][:, j, :],
                                    kT_sb[psl, hp, 128 * kb : 128 * kb + 128],
                                    qT_sb[psl, hp, qsl],
                                    start=True,
                                    stop=True,
                                )
                        for m in range(2):
                            nc.scalar.activation(
                                out=p_sb[m],
                                in_=s_ps[m],
                                func=mybir.ActivationFunctionType.Exp,
                            )
                        if causal:
                            for j in range(2):
                                kb = 2 * grp + j
                                if kb >= 4 * g:  # diagonal straddle: mask k > q
                                    jm = kb - 4 * g
                                    msl = slice(384 - 128 * jm, 896 - 128 * jm)
                                    for m in range(2):
                                        nc.vector.tensor_mul(
                                            out=p_sb[m][:, j, :],
                                            in0=p_sb[m][:, j, :],
                                            in1=mask_sb[:, msl],
                                        )
                        for j in range(2):
                            kb = 2 * grp + j
                            first = kb == 0
                            last = kb == nkb - 1
                            for m in range(2):
                                # [V_h | 1] stationary: rows 0..63 ctx^T,
                                # row 64 the softmax denominator
                                nc.tensor.matmul(
                                    ctx_ps[m][0 : DK + 1, :],
                                    v_sb[:, kb, 2 * hp + m, :],
                                    p_sb[m][:, j, :],
                                    start=first,
                                    stop=last,
                                )
                    # normalize: ctx^T rows *= 1/den (den = ctx row 64)
                    for m in range(2):
                        rr = rbuf.tile([128, 512], F32R, tag="rr")
                        with nc.allow_low_precision(
                            reason="f32r is fp32 storage; recip is elementwise"
                        ):
                            nc.vector.reciprocal(
                                out=rr[DK : DK + 1, :],
                                in_=ctx_ps[m][DK : DK + 1, :],
                            )
                        r_ps = ps_s.tile([128, 2, 512], F32, tag="s", name="r_ps")
                        nc.tensor.matmul(
                            r_ps[0:DK, 0, :],
                            ones_sb[DK : DK + 1, :],
                            rr[DK : DK + 1, :],
                            start=True,
                            stop=True,
                        )
                        r_sb = rbuf.tile([64, 512], F32, tag="r_sb")
                        nc.vector.tensor_copy(out=r_sb, in_=r_ps[0:DK, 0, :])
                        nc.vector.tensor_mul(
                            out=ctx_sb[64 * m : 64 * m + 64, hp, qsl],
                            in0=ctx_ps[m][0:DK, :],
                            in1=r_sb,
                        )

            # ================= Phase O: out projection =================
            wo_sb = wts.tile([128, NOB, D], F32R, tag="w")
            nc.sync.dma_start(
                out=wo_sb, in_=woT[:, :].rearrange("(cb p) o -> p cb o", p=128)
            )
            for tb in range(S // 128):
                for on in range(2):
                    ps = ps_proj.tile([128, 512], F32, tag="proj")
                    for cb in range(NOB):
                        nc.tensor.matmul(
                            ps,
                            ctx_sb[:, cb, 128 * tb : 128 * tb + 128],
                            wo_sb[:, cb, 512 * on : 512 * on + 512],
                            start=(cb == 0),
                            stop=(cb == NOB - 1),
                        )
                    o_sb = outst.tile([128, 512], F32, tag="o")
                    nc.vector.tensor_copy(out=o_sb, in_=ps)
                    nc.sync.dma_start(
                        out=out[128 * tb : 128 * tb + 128, 512 * on : 512 * on + 512],
                        in_=o_sb,
                    )

    nc.compile()
    return nc


_PROG_CACHE: dict = {}


def _get_program(causal: bool):
    key = ("mha", causal)
    if key not in _PROG_CACHE:
        _PROG_CACHE[key] = build_mha_program(causal)
    return _PROG_CACHE[key]


def _make_maskT() -> np.ndarray:
    kk = np.arange(128)[:, None]
    c = np.arange(896)[None, :]
    return (c >= kk + 384).astype(np.float32)


def kernel(q, k, v, Wq, bq, Wk, bk, Wv, bv, Wo, bo, attn_mask, padding_mask,
           trace=False):
    q = np.asarray(q, np.float32)
    k = np.asarray(k, np.float32)
    v = np.asarray(v, np.float32)
    Wq = np.asarray(Wq, np.float32)
    bq = np.asarray(bq, np.float32)
    Wk = np.asarray(Wk, np.float32)
    bk = np.asarray(bk, np.float32)
    Wv = np.asarray(Wv, np.float32)
    bv = np.asarray(bv, np.float32)
    Wo = np.asarray(Wo, np.float32)
    bo = np.asarray(bo, np.float32)
    am = np.asarray(attn_mask)
    pm = np.asarray(padding_mask)

    if not (pm != 0).all():
        raise NotImplementedError("padding_mask with zeros not supported yet")
    tril = np.tril(np.ones((S, S), np.int32))
    if (am != 0).all():
        causal = False
    elif ((am != 0) == (tril != 0)).all():
        causal = True
    else:
        raise NotImplementedError("general attn_mask not supported yet")

    nc = _get_program(causal)
    scale = 1.0 / 8.0  # 1/sqrt(DK)
    maskT = _make_maskT()
    onesf = np.ones((128, 64), np.float32)

    in_maps = []
    for core in range(NCORE):
        b, g = divmod(core, 2)
        osl = slice(O * g, O * g + O)
        in_maps.append({
            "xqT": np.ascontiguousarray(q[b].T),
            "xkT": np.ascontiguousarray(k[b].T),
            "xvT": np.ascontiguousarray(v[b].T),
            "wqT": np.ascontiguousarray((Wq[osl] * scale).T),
            "wkT": np.ascontiguousarray(Wk[osl].T),
            "wvT": np.ascontiguousarray(Wv[osl].T),
            "woT": np.ascontiguousarray(Wo[:, osl].T),
            "bq2": np.ascontiguousarray((bq[osl] * scale).reshape(NOB, 128).T),
            "bk2": np.ascontiguousarray(bk[osl].reshape(NOB, 128).T),
            "maskT": maskT,
            "onesd": onesf,
        })

    res = run_bass_kernel_spmd(
        nc, in_maps, core_ids=list(range(NCORE)), trace=trace
    )

    bias_total = (Wo @ bv + bo).astype(np.float32)
    outp = np.empty((B, S, D), np.float32)
    for b in range(B):
        outp[b] = (
            res.results[2 * b]["out"]
            + res.results[2 * b + 1]["out"]
            + bias_total[None, :]
        )
    if trace:
        kernel._last_results = res
    return outp
